# revision 18
# baseline (speedup 1.0000x reference)
"""Trainium2 Bass kernel for nn_MemoryUnit (scatter_memory).

Computes, for x = input + pos_embedding, rows r = (b,h,w), memory W [2000,256]:
  att   = softmax(x_r . W_m)  over m
  me    = att * 1[att > SHRINK]          (hard-shrink, L1-renormalized)
  out_r = (me @ W) / sum_m(me)
  compact_loss  = mean((x - W[argmax att])^2)
  distance_loss = sum_{i<j} relu(1 - ||w_i - w_j||^2) * 2 / (m(m-1))

Strategy (8 NeuronCores, data-parallel over batch, 4 batches/core):
  Layout B everywhere: memory slots m on SBUF partitions, rows on the free
  axis.  The native [b, c, h, w] input layout is exactly the transposed
  [c, rows] operand the TensorEngine wants, and the output [c, rows] psum
  tiles DMA straight back into [b, c, h, w] -- zero transposes.

  Precision: the hard-shrink keeps only ~17 of 2000 slots per row, so the
  L1 renormalization amplifies any threshold flip; logits must be ~fp32.
  mm1 runs as 3 fp16 passes (W_hi@x_hi + W_hi@x_lo + W_lo@x_hi, fp32 psum
  accumulate) which matches fp32 logits to ~1e-6.  exp stays fp32 through
  the threshold compare; the masked weights and mm2 run in fp16
  (measured end-to-end output rel err ~3e-4).

  Per 512-row chunk: mm1 -> exp(fp32) + exp(fp16 copy) -> s' = sum_m e via
  ones-matmul -> thr = SHRINK*s' broadcast (DRAM-bounce DMA) -> mask (DVE,
  fp32 cmp) -> me = e*mask (fp16, + per-slot row-sum accum for the loss) ->
  sm ones-matmul + mm2 (fp16) -> out = mm2/sm -> DMA out.  max_m e for
  compact_loss via DVE pairwise-max tree + PE transpose + free-axis max.
  sq[argmax] is approximated by the me-weighted mean of ||w_m||^2 (the term
  is 0.07% of compact_loss; approximation error ~1e-5 relative).
  Scalar-loss partial sums are returned per-core and combined on host.
"""

import sys

for _p in ("/opt/trn_rl_repo", "/opt/trn_rl_repo/concourse"):
    if _p not in sys.path:
        sys.path.insert(0, _p)

import numpy as np
import ml_dtypes

# ---- problem constants (hardcoded per contract) ----
B = 32          # batch
C = 256         # feature dim
HW = 1024       # fmap*fmap
M = 2000        # memory slots
SHRINK = 0.0025
NCORES = 8
BPC = B // NCORES          # batches per core = 4
ROWS = BPC * HW            # rows per core = 4096
R = 512                    # rows per chunk
NCHUNK = ROWS // R         # 8
MT = 16                    # m tiles
MSZ = [128] * 15 + [80]    # m tile sizes (15*128+80 = 2000)
MJ = M // NCORES           # distance-loss column slice per core = 250
NTOT = B * HW              # 32768 global rows

_BF16 = ml_dtypes.bfloat16
_F16 = np.float16

_CACHE = {}


def _build():
    """Build the Bass/Tile SPMD program (same program on all 8 cores)."""
    import concourse.bass as bass
    import concourse.mybir as mybir
    import concourse.tile as tile

    fp32 = mybir.dt.float32
    f16 = mybir.dt.float16
    Alu = mybir.AluOpType
    Act = mybir.ActivationFunctionType

    nc = bass.Bass()

    # ---- DRAM I/O (per core) ----
    x_in = nc.dram_tensor("x_in", [BPC, C, HW], fp32, kind="ExternalInput")
    pos_in = nc.dram_tensor("pos_in", [C, HW], fp32, kind="ExternalInput")
    wth_in = nc.dram_tensor("wth_in", [C, M], f16, kind="ExternalInput")   # hi(W^T)
    wtl_in = nc.dram_tensor("wtl_in", [C, M], f16, kind="ExternalInput")   # lo(W^T)
    w_in = nc.dram_tensor("w_in", [M, C], f16, kind="ExternalInput")       # W (mm2)
    wtj_in = nc.dram_tensor("wtj_in", [C, MJ], f16, kind="ExternalInput")  # W^T cols
    sqj_in = nc.dram_tensor("sqj_in", [1, MJ], fp32, kind="ExternalInput")
    b1msq_in = nc.dram_tensor("b1msq_in", [128, MT], fp32, kind="ExternalInput")
    id_in = nc.dram_tensor("id_in", [128, 128], f16, kind="ExternalInput")

    y_out = nc.dram_tensor("y_out", [BPC, C, HW], fp32, kind="ExternalOutput")
    mrs_out = nc.dram_tensor("mrs_out", [128, MT * NCHUNK], fp32, kind="ExternalOutput")
    fsum_out = nc.dram_tensor("fsum_out", [128, MT], fp32, kind="ExternalOutput")
    xsq_out = nc.dram_tensor("xsq_out", [128, 2 * NCHUNK], fp32, kind="ExternalOutput")
    mxl_out = nc.dram_tensor("mxl_out", [128, NCHUNK], fp32, kind="ExternalOutput")

    def bcast_ap(dram_ap, parts=128):
        """DRAM AP read with partition-stride 0 -> broadcast to `parts` partitions."""
        return bass.AP(
            tensor=dram_ap.tensor,
            offset=dram_ap.offset,
            ap=[[0, parts]] + list(dram_ap.ap),
        )

    from contextlib import ExitStack

    with ExitStack() as ctx:
        tc = ctx.enter_context(tile.TileContext(nc))
        const = ctx.enter_context(tc.tile_pool(name="const", bufs=1))
        xpool = ctx.enter_context(tc.tile_pool(name="xpool", bufs=2))
        epool = ctx.enter_context(tc.tile_pool(name="epool", bufs=1))
        e16pool = ctx.enter_context(tc.tile_pool(name="e16pool", bufs=1))
        mpool = ctx.enter_context(tc.tile_pool(name="mpool", bufs=3))
        mepool = ctx.enter_context(tc.tile_pool(name="mepool", bufs=4))
        tpool = ctx.enter_context(tc.tile_pool(name="tpool", bufs=1))
        bpool = ctx.enter_context(tc.tile_pool(name="bpool", bufs=2))
        ypool = ctx.enter_context(tc.tile_pool(name="ypool", bufs=3))
        pl = ctx.enter_context(tc.tile_pool(name="pl", bufs=2, space="PSUM"))
        ps = ctx.enter_context(tc.tile_pool(name="ps", bufs=2, space="PSUM"))
        psm = ctx.enter_context(tc.tile_pool(name="psm", bufs=1, space="PSUM"))
        po = ctx.enter_context(tc.tile_pool(name="po", bufs=1, space="PSUM"))
        pt = ctx.enter_context(tc.tile_pool(name="pt", bufs=1, space="PSUM"))
        dscr = ctx.enter_context(tc.tile_pool(name="dscr", bufs=2, space="DRAM"))

        # ---- constants into SBUF ----
        wth_sb, wtl_sb = [], []
        for cc in range(2):
            th_ = const.tile([128, M], f16, tag=f"wth{cc}", name=f"wth{cc}")
            nc.sync.dma_start(out=th_[:], in_=wth_in[cc * 128:(cc + 1) * 128, :])
            wth_sb.append(th_)
            tl_ = const.tile([128, M], f16, tag=f"wtl{cc}", name=f"wtl{cc}")
            nc.sync.dma_start(out=tl_[:], in_=wtl_in[cc * 128:(cc + 1) * 128, :])
            wtl_sb.append(tl_)
        w_sb = const.tile([128, MT * C], f16, tag="w_sb")
        for t in range(MT):
            nc.sync.dma_start(
                out=w_sb[:MSZ[t], t * C:(t + 1) * C],
                in_=w_in[t * 128:t * 128 + MSZ[t], :],
            )
        pos_sb = []
        for cc in range(2):
            t = const.tile([128, HW], fp32, tag=f"pos{cc}", name=f"pos{cc}")
            nc.sync.dma_start(out=t[:], in_=pos_in[cc * 128:(cc + 1) * 128, :])
            pos_sb.append(t)
        wtj_sb = []
        for cc in range(2):
            t = const.tile([128, MJ], f16, tag=f"wtj{cc}", name=f"wtj{cc}")
            nc.sync.dma_start(out=t[:], in_=wtj_in[cc * 128:(cc + 1) * 128, :])
            wtj_sb.append(t)
        b1msq = const.tile([128, MT], fp32, tag="b1msq")
        nc.sync.dma_start(out=b1msq[:], in_=b1msq_in[:, :])
        id_sb = const.tile([128, 128], f16, tag="id_sb")
        nc.sync.dma_start(out=id_sb[:], in_=id_in[:, :])

        ones_sb = const.tile([128, 1], f16, tag="ones")
        nc.vector.memset(ones_sb[:], 1.0)

        # stats accumulators
        mrs = const.tile([128, MT * NCHUNK], fp32, tag="mrs")
        nc.gpsimd.memset(mrs[:], 0.0)
        fsum = const.tile([128, MT], fp32, tag="fsum")
        nc.gpsimd.memset(fsum[:], 0.0)
        xsq = const.tile([128, 2 * NCHUNK], fp32, tag="xsq")
        nc.gpsimd.memset(xsq[:], 0.0)
        mxl = const.tile([128, NCHUNK], fp32, tag="mxl")
        nc.gpsimd.memset(mxl[:], 0.0)

        # ---- distance loss: G = W @ W^T column-slice, f = relu(1 - d2) ----
        bsqj = const.tile([128, MJ], fp32, tag="bsqj")
        nc.sync.dma_start(out=bsqj[:], in_=bcast_ap(sqj_in[0, :]))
        for mi in range(MT):
            msz = MSZ[mi]
            pg = pl.tile([128, R], mybir.dt.float32, tag="pl", name="pg")
            for cc in range(2):
                nc.tensor.matmul(
                    pg[:msz, :MJ],
                    lhsT=wth_sb[cc][:, mi * 128:mi * 128 + msz],
                    rhs=wtj_sb[cc][:, :],
                    start=(cc == 0),
                    stop=(cc == 1),
                )
            u = xpool.tile([128, R], mybir.dt.float32, tag="dist_u", name="u")
            nc.vector.scalar_tensor_tensor(
                out=u[:msz, :MJ],
                in0=pg[:msz, :MJ],
                scalar=2.0,
                in1=bsqj[:msz, :],
                op0=Alu.mult,
                op1=Alu.subtract,
            )
            fscr = xpool.tile([128, R], f16, tag="dist_f", name="fscr")
            nc.scalar.activation(
                out=fscr[:msz, :MJ],
                in_=u[:msz, :MJ],
                func=Act.Relu,
                bias=b1msq[:msz, mi:mi + 1],
                scale=1.0,
                accum_out=fsum[:msz, mi:mi + 1],
            )

        # ---- main pipeline ----
        def phase_a(k):
            """DMA + x-prep + mm1 (split-3 fp16) + exp + s' ones-matmul."""
            b, h = k // 2, k % 2
            xh, xl = [], []
            for cc in range(2):
                xin = xpool.tile([128, R], mybir.dt.float32, tag=f"xin{cc}", name="xin")
                nc.sync.dma_start(
                    out=xin[:],
                    in_=x_in[b, cc * 128:(cc + 1) * 128, h * R:(h + 1) * R],
                )
                xf = xpool.tile([128, R], mybir.dt.float32, tag=f"xf{cc}", name="xf")
                nc.vector.tensor_add(xf[:], xin[:], pos_sb[cc][:, h * R:(h + 1) * R])
                xht = xpool.tile([128, R], f16, tag=f"xh{cc}", name="xht")
                nc.scalar.copy(out=xht[:], in_=xf[:])
                xlt = xpool.tile([128, R], f16, tag=f"xl{cc}", name="xlt")
                nc.vector.tensor_sub(xlt[:], xf[:], xht[:])
                sqs = xpool.tile([128, R], f16, tag=f"xsqs{cc}", name="sqs")
                nc.scalar.activation(
                    out=sqs[:], in_=xf[:], func=Act.Square,
                    accum_out=xsq[:, 2 * k + cc:2 * k + cc + 1],
                )
                xh.append(xht)
                xl.append(xlt)

            ps_t = ps.tile([128, R], mybir.dt.float32, tag="ps", name="ps_t")
            efs = []
            e16s = []
            for t in range(MT):
                msz = MSZ[t]
                plt = pl.tile([128, R], mybir.dt.float32, tag="pl", name="plt")
                passes = [
                    (wth_sb, xh), (wth_sb, xl), (wtl_sb, xh),
                ]
                np_ = len(passes) * 2
                i = 0
                for wt_list, x_list in passes:
                    for cc in range(2):
                        nc.tensor.matmul(
                            plt[:msz, :],
                            lhsT=wt_list[cc][:, t * 128:t * 128 + msz],
                            rhs=x_list[cc][:],
                            start=(i == 0),
                            stop=(i == np_ - 1),
                        )
                        i += 1
                ef = epool.tile([128, R], mybir.dt.float32, tag=f"ef{t}", name="ef")
                nc.scalar.activation(out=ef[:msz, :], in_=plt[:msz, :], func=Act.Exp)
                e16 = e16pool.tile([128, R], f16, tag=f"e16_{t}", name="e16")
                if msz < 128:
                    # zero tail partitions so the max-tree can read all 128
                    nc.gpsimd.memset(e16[64:128, :], 0.0)
                nc.scalar.activation(out=e16[:msz, :], in_=plt[:msz, :], func=Act.Exp)
                nc.tensor.matmul(
                    ps_t[0:1, :],
                    lhsT=ones_sb[:msz, :],
                    rhs=e16[:msz, :],
                    start=(t == 0),
                    stop=(t == MT - 1),
                )
                efs.append(ef)
                e16s.append(e16)
            return ps_t, efs, e16s

        def phase_b(k, ps_t, efs, e16s):
            """Threshold, shrink, mm2, max-tree, scale, DMA-out for chunk k."""
            b, h = k // 2, k % 2
            thr = bpool.tile([1, R], mybir.dt.float32, tag="thr")
            nc.scalar.mul(out=thr[0:1, :], in_=ps_t[0:1, :], mul=SHRINK)
            thr_d = dscr.tile([1, R], mybir.dt.float32, tag="thr_d", space="DRAM")
            nc.sync.dma_start(out=thr_d[0, :], in_=thr[0:1, :])
            b_t = bpool.tile([128, R], mybir.dt.float32, tag="b_t")
            nc.sync.dma_start(out=b_t[:], in_=bcast_ap(thr_d[0, :]))

            psm_t = psm.tile([128, R], mybir.dt.float32, tag="psm", name="psm_t")
            po_t = [
                po.tile([128, R], mybir.dt.float32, tag=f"po{cc}", name=f"po{cc}")
                for cc in range(2)
            ]
            for t in range(MT):
                msz = MSZ[t]
                ef = efs[t]
                mask = mpool.tile([128, R], f16, tag="mask", name="mask")
                nc.vector.tensor_tensor(mask[:msz, :], ef[:msz, :], b_t[:msz, :], op=Alu.is_gt)
                me = mepool.tile([128, R], f16, tag="me", name="me")
                nc.vector.scalar_tensor_tensor(
                    out=me[:msz, :],
                    in0=ef[:msz, :],
                    scalar=1.0,
                    in1=mask[:msz, :],
                    op0=Alu.mult,
                    op1=Alu.mult,
                    accum_out=mrs[:msz, t * NCHUNK + k:t * NCHUNK + k + 1],
                )
                nc.tensor.matmul(
                    psm_t[0:1, :],
                    lhsT=ones_sb[:msz, :],
                    rhs=me[:msz, :],
                    start=(t == 0),
                    stop=(t == MT - 1),
                )
                for cc in range(2):
                    nc.tensor.matmul(
                        po_t[cc][:, :],
                        lhsT=w_sb[:msz, t * C + cc * 128:t * C + (cc + 1) * 128],
                        rhs=me[:msz, :],
                        start=(t == 0),
                        stop=(t == MT - 1),
                    )

            # max over m: DVE pairwise-max tree, then PE-transpose 128-row
            # blocks + free-axis max -> per-row max e -> mean(log(max e))
            lvl = []
            for i in range(8):
                mx = tpool.tile([128, R], f16, tag=f"tr0_{i}", name="mx")
                nc.vector.tensor_tensor(mx[:], e16s[2 * i][:], e16s[2 * i + 1][:], op=Alu.max)
                lvl.append(mx)
            while len(lvl) > 1:
                nxt = []
                for i in range(len(lvl) // 2):
                    mx = tpool.tile([128, R], f16, tag=f"tr{len(lvl)}_{i}", name="mx")
                    nc.vector.tensor_tensor(mx[:], lvl[2 * i][:], lvl[2 * i + 1][:], op=Alu.max)
                    nxt.append(mx)
                lvl = nxt
            mx4 = bpool.tile([128, R // 128], mybir.dt.float32, tag="mx4")
            for j in range(R // 128):
                ptt = pt.tile([128, 128], f16, tag="ptt", name="ptt")
                nc.tensor.transpose(ptt[:], lvl[0][:, j * 128:(j + 1) * 128], id_sb[:])
                nc.vector.tensor_reduce(
                    mx4[:, j:j + 1], ptt[:], axis=mybir.AxisListType.X, op=Alu.max
                )
            lnscr = bpool.tile([128, R // 128], mybir.dt.float32, tag="lnscr")
            nc.scalar.activation(out=lnscr[:], in_=mx4[:], func=Act.Ln,
                                 accum_out=mxl[:, k:k + 1])

            # 1/sm with zero-row guard, broadcast, scale, store
            smg = bpool.tile([1, R], mybir.dt.float32, tag="smg")
            nc.vector.tensor_scalar_max(smg[0:1, :], psm_t[0:1, :], 1e-30)
            rsm = bpool.tile([1, R], mybir.dt.float32, tag="rsm")
            nc.vector.reciprocal(rsm[0:1, :], smg[0:1, :])
            rsm_d = dscr.tile([1, R], mybir.dt.float32, tag="rsm_d", space="DRAM")
            nc.sync.dma_start(out=rsm_d[0, :], in_=rsm[0:1, :])
            b_r = bpool.tile([128, R], mybir.dt.float32, tag="b_r")
            nc.sync.dma_start(out=b_r[:], in_=bcast_ap(rsm_d[0, :]))
            for cc in range(2):
                yt = ypool.tile([128, R], mybir.dt.float32, tag=f"yt{cc}", name="yt")
                nc.vector.tensor_tensor(yt[:], po_t[cc][:], b_r[:], op=Alu.mult)
                nc.sync.dma_start(
                    out=y_out[b, cc * 128:(cc + 1) * 128, h * R:(h + 1) * R],
                    in_=yt[:],
                )

        # software pipeline: A(0), A(1), B(0), A(2), B(1), ..., B(7)
        pending = phase_a(0)
        for k in range(1, NCHUNK):
            nxt = phase_a(k)
            phase_b(k - 1, *pending)
            pending = nxt
        phase_b(NCHUNK - 1, *pending)

        # stats out
        nc.sync.dma_start(out=mrs_out[:, :], in_=mrs[:])
        nc.sync.dma_start(out=fsum_out[:, :], in_=fsum[:])
        nc.sync.dma_start(out=xsq_out[:, :], in_=xsq[:])
        nc.sync.dma_start(out=mxl_out[:, :], in_=mxl[:])

    _split_multiwaits(nc, mybir)
    return nc


def _split_multiwaits(nc, mybir):
    """This walrus build accepts at most ONE sync wait per instruction; Tile
    attaches several.  Move extra waits onto injected same-engine NOPs."""
    n_split = 0
    dma_multi = []
    for fn in nc.m.functions:
        for bb in fn.blocks:
            out = []
            for inst in bb.instructions:
                si = getattr(inst, "sync_info", None)
                ow = list(si.on_wait) if si and si.on_wait else []
                is_dma = type(inst).__name__ in (
                    "InstTensorCopy", "InstTensorLoad", "InstTensorSave"
                )
                if len(ow) > 1 and not is_dma:
                    for w in ow[:-1]:
                        out.append(mybir.InstNoOp(
                            name=nc.get_next_instruction_name(),
                            ins=[], outs=[],
                            engine=inst.engine,
                            sync_info=mybir.SyncInfo(on_wait=[w], on_update=[]),
                        ))
                        n_split += 1
                    inst.sync_info = mybir.SyncInfo(
                        on_wait=[ow[-1]],
                        on_update=list(si.on_update) if si.on_update else [],
                    )
                elif len(ow) > 1:
                    dma_multi.append((inst.name, [w.ant_name for w in ow]))
                out.append(inst)
            bb.instructions = out
    if dma_multi:
        raise RuntimeError(f"multi-wait DMA instructions present: {dma_multi[:8]}")
    return n_split


def _get_program():
    if "nc" not in _CACHE:
        _CACHE["nc"] = _build()
    return _CACHE["nc"]


def _prep_inputs(input, weight, pos_embedding):
    """Shard the full inputs into per-core in_maps."""
    x = np.asarray(input, dtype=np.float32).reshape(B, C, HW)
    pos = np.asarray(pos_embedding, dtype=np.float32).reshape(C, HW)
    w = np.asarray(weight, dtype=np.float32)

    wt = np.ascontiguousarray(w.T)                          # [C, M] fp32
    wth = wt.astype(_F16)
    wtl = (wt - wth.astype(np.float32)).astype(_F16)
    wf16 = w.astype(_F16)                                   # [M, C]
    sq = (w.astype(np.float32) ** 2).sum(axis=1)            # [M] fp32
    sqpad = np.zeros(MT * 128, dtype=np.float32)
    sqpad[:M] = sq
    b1msq = np.ascontiguousarray(
        (1.0 - sqpad).reshape(MT, 128).T
    ).astype(np.float32)                                    # [128, MT]

    in_maps = []
    for k in range(NCORES):
        in_maps.append({
            "x_in": np.ascontiguousarray(x[BPC * k:BPC * (k + 1)]),
            "pos_in": pos,
            "wth_in": wth,
            "wtl_in": wtl,
            "w_in": wf16,
            "wtj_in": np.ascontiguousarray(wth[:, MJ * k:MJ * (k + 1)]),
            "sqj_in": np.ascontiguousarray(sq[MJ * k:MJ * (k + 1)]).reshape(1, MJ),
            "b1msq_in": b1msq,
            "id_in": np.eye(128, dtype=_F16),
        })
    return in_maps, sq


def _combine(results, sq):
    """Gather per-core outputs into full tensors + scalar losses."""
    out = np.concatenate([r["y_out"] for r in results], axis=0)  # [B, C, HW]
    out = out.reshape(B, C, 32, 32)

    mrs = np.zeros((128, MT), dtype=np.float64)
    fsum_tot = 0.0
    xsq_tot = 0.0
    mxl_tot = 0.0
    for r in results:
        mrs += r["mrs_out"].astype(np.float64).reshape(128, MT, NCHUNK).sum(axis=2)
        fsum_tot += r["fsum_out"].astype(np.float64).sum()
        xsq_tot += r["xsq_out"].astype(np.float64).sum()
        mxl_tot += r["mxl_out"].astype(np.float64).sum()

    # sq[argmax] approximated by me-weighted mean of sq
    sqpad = np.zeros(MT * 128, dtype=np.float64)
    sqpad[:M] = sq
    sqt = sqpad.reshape(MT, 128).T                            # [128, MT]
    num = float((mrs * sqt).sum())
    den = float(mrs.sum())
    sqidx = num / max(den, 1e-300)

    compact = (xsq_tot - 2.0 * mxl_tot + NTOT * sqidx) / (NTOT * C)
    distance = (fsum_tot - float(M)) / (float(M) * (M - 1))

    return out.astype(np.float32), np.float32(compact), np.float32(distance)


def kernel(input, label_batch=None, weight=None, pos_embedding=None, **_):
    from concourse.bass_utils import run_bass_kernel_spmd

    nc = _get_program()
    in_maps, sq = _prep_inputs(input, weight, pos_embedding)
    res = run_bass_kernel_spmd(nc, in_maps, core_ids=list(range(NCORES)))
    return _combine(res.results, sq)


if __name__ == "__main__":
    rng = np.random.default_rng(0)
    inp = {
        "input": rng.standard_normal((B, C, 32, 32), dtype=np.float32),
        "label_batch": rng.integers(0, 2, (B,)),
        "weight": ((rng.random((M, C), dtype=np.float32) - 0.5) / 8.0),
        "pos_embedding": rng.standard_normal((1, C, 32, 32), dtype=np.float32),
    }
    out, cl, dl = kernel(**inp)
    print(out.shape, cl, dl)


# revision 23
# speedup vs baseline: 44.9020x; 44.9020x over previous
"""Trainium2 Bass kernel for nn_MemoryUnit (scatter_memory).

Computes, for x = input + pos_embedding, rows r = (b,h,w), memory W [2000,256]:
  att   = softmax(x_r . W_m)  over m
  me    = att * 1[att > SHRINK]          (hard-shrink, L1-renormalized)
  out_r = (me @ W) / sum_m(me)
  compact_loss  = mean((x - W[argmax att])^2)
  distance_loss = sum_{i<j} relu(1 - ||w_i - w_j||^2) * 2 / (m(m-1))

Strategy (8 NeuronCores, data-parallel over batch, 4 batches/core):
  Layout B everywhere: memory slots m on SBUF partitions, rows on the free
  axis.  The native [b, c, h, w] input layout is exactly the transposed
  [c, rows] operand the TensorEngine wants, and the output [c, rows] psum
  tiles DMA straight back into [b, c, h, w] -- zero transposes.

  Precision: the hard-shrink keeps only ~17 of 2000 slots per row, so the
  L1 renormalization amplifies any threshold flip; logits must be ~fp32.
  mm1 runs as 3 fp16 passes (W_hi@x_hi + W_hi@x_lo + W_lo@x_hi, fp32 psum
  accumulate) which matches fp32 logits to ~1e-6.  exp stays fp32 through
  the threshold compare; the masked weights and mm2 run in fp16
  (measured end-to-end output rel err ~3e-4).

  Per 512-row chunk: mm1 -> exp(fp32) + exp(fp16 copy) -> s' = sum_m e via
  ones-matmul -> thr = SHRINK*s' broadcast (DRAM-bounce DMA) -> mask (DVE,
  fp32 cmp) -> me = e*mask (fp16, + per-slot row-sum accum for the loss) ->
  sm ones-matmul + mm2 (fp16) -> out = mm2/sm -> DMA out.  max_m e for
  compact_loss via DVE pairwise-max tree + PE transpose + free-axis max.
  sq[argmax] is approximated by the me-weighted mean of ||w_m||^2 (the term
  is 0.07% of compact_loss; approximation error ~1e-5 relative).
  Scalar-loss partial sums are returned per-core and combined on host.
"""

import sys

for _p in ("/opt/trn_rl_repo", "/opt/trn_rl_repo/concourse"):
    if _p not in sys.path:
        sys.path.insert(0, _p)

import numpy as np
import ml_dtypes

# ---- problem constants (hardcoded per contract) ----
B = 32          # batch
C = 256         # feature dim
HW = 1024       # fmap*fmap
M = 2000        # memory slots
SHRINK = 0.0025
NCORES = 8
BPC = B // NCORES          # batches per core = 4
ROWS = BPC * HW            # rows per core = 4096
R = 512                    # rows per chunk
NCHUNK = ROWS // R         # 8
MT = 16                    # m tiles
MSZ = [128] * 15 + [80]    # m tile sizes (15*128+80 = 2000)
MJ = M // NCORES           # distance-loss column slice per core = 250
NTOT = B * HW              # 32768 global rows

_BF16 = ml_dtypes.bfloat16
_F16 = np.float16

_CACHE = {}


def _build():
    """Build the Bass/Tile SPMD program (same program on all 8 cores)."""
    import concourse.bass as bass
    import concourse.mybir as mybir
    import concourse.tile as tile

    fp32 = mybir.dt.float32
    f16 = mybir.dt.float16
    Alu = mybir.AluOpType
    Act = mybir.ActivationFunctionType

    nc = bass.Bass()

    # ---- DRAM I/O (per core) ----
    x_in = nc.dram_tensor("x_in", [BPC, C, HW], fp32, kind="ExternalInput")
    pos_in = nc.dram_tensor("pos_in", [C, HW], fp32, kind="ExternalInput")
    wth_in = nc.dram_tensor("wth_in", [C, M], f16, kind="ExternalInput")   # hi(W^T)
    wtl_in = nc.dram_tensor("wtl_in", [C, M], f16, kind="ExternalInput")   # lo(W^T)
    w_in = nc.dram_tensor("w_in", [M, C], f16, kind="ExternalInput")       # W (mm2)
    wtj_in = nc.dram_tensor("wtj_in", [C, MJ], f16, kind="ExternalInput")  # W^T cols
    sqj_in = nc.dram_tensor("sqj_in", [1, MJ], fp32, kind="ExternalInput")
    b1msq_in = nc.dram_tensor("b1msq_in", [128, MT], fp32, kind="ExternalInput")
    id_in = nc.dram_tensor("id_in", [128, 128], f16, kind="ExternalInput")

    y_out = nc.dram_tensor("y_out", [BPC, C, HW], fp32, kind="ExternalOutput")
    mrs_out = nc.dram_tensor("mrs_out", [128, MT * NCHUNK], fp32, kind="ExternalOutput")
    fsum_out = nc.dram_tensor("fsum_out", [128, MT], fp32, kind="ExternalOutput")
    xsq_out = nc.dram_tensor("xsq_out", [128, 2 * NCHUNK], fp32, kind="ExternalOutput")
    mxl_out = nc.dram_tensor("mxl_out", [128, NCHUNK], fp32, kind="ExternalOutput")

    def bcast_ap(dram_ap, parts=128):
        """DRAM AP read with partition-stride 0 -> broadcast to `parts` partitions."""
        return bass.AP(
            tensor=dram_ap.tensor,
            offset=dram_ap.offset,
            ap=[[0, parts]] + list(dram_ap.ap),
        )

    from contextlib import ExitStack

    with ExitStack() as ctx:
        tc = ctx.enter_context(tile.TileContext(nc))
        const = ctx.enter_context(tc.tile_pool(name="const", bufs=1))
        xpool = ctx.enter_context(tc.tile_pool(name="xpool", bufs=2))
        epool = ctx.enter_context(tc.tile_pool(name="epool", bufs=1))
        e16pool = ctx.enter_context(tc.tile_pool(name="e16pool", bufs=1))
        mpool = ctx.enter_context(tc.tile_pool(name="mpool", bufs=3))
        mepool = ctx.enter_context(tc.tile_pool(name="mepool", bufs=4))
        tpool = ctx.enter_context(tc.tile_pool(name="tpool", bufs=1))
        bpool = ctx.enter_context(tc.tile_pool(name="bpool", bufs=2))
        ypool = ctx.enter_context(tc.tile_pool(name="ypool", bufs=3))
        pl = ctx.enter_context(tc.tile_pool(name="pl", bufs=2, space="PSUM"))
        ps = ctx.enter_context(tc.tile_pool(name="ps", bufs=2, space="PSUM"))
        psm = ctx.enter_context(tc.tile_pool(name="psm", bufs=1, space="PSUM"))
        po = ctx.enter_context(tc.tile_pool(name="po", bufs=1, space="PSUM"))
        pt = ctx.enter_context(tc.tile_pool(name="pt", bufs=1, space="PSUM"))
        dscr = ctx.enter_context(tc.tile_pool(name="dscr", bufs=2, space="DRAM"))

        # ---- constants into SBUF ----
        wth_sb, wtl_sb = [], []
        for cc in range(2):
            th_ = const.tile([128, M], f16, tag=f"wth{cc}", name=f"wth{cc}")
            nc.sync.dma_start(out=th_[:], in_=wth_in[cc * 128:(cc + 1) * 128, :])
            wth_sb.append(th_)
            tl_ = const.tile([128, M], f16, tag=f"wtl{cc}", name=f"wtl{cc}")
            nc.sync.dma_start(out=tl_[:], in_=wtl_in[cc * 128:(cc + 1) * 128, :])
            wtl_sb.append(tl_)
        w_sb = const.tile([128, MT * C], f16, tag="w_sb")
        for t in range(MT):
            nc.sync.dma_start(
                out=w_sb[:MSZ[t], t * C:(t + 1) * C],
                in_=w_in[t * 128:t * 128 + MSZ[t], :],
            )
        pos_sb = []
        for cc in range(2):
            t = const.tile([128, HW], fp32, tag=f"pos{cc}", name=f"pos{cc}")
            nc.sync.dma_start(out=t[:], in_=pos_in[cc * 128:(cc + 1) * 128, :])
            pos_sb.append(t)
        wtj_sb = []
        for cc in range(2):
            t = const.tile([128, MJ], f16, tag=f"wtj{cc}", name=f"wtj{cc}")
            nc.sync.dma_start(out=t[:], in_=wtj_in[cc * 128:(cc + 1) * 128, :])
            wtj_sb.append(t)
        b1msq = const.tile([128, MT], fp32, tag="b1msq")
        nc.sync.dma_start(out=b1msq[:], in_=b1msq_in[:, :])
        id_sb = const.tile([128, 128], f16, tag="id_sb")
        nc.sync.dma_start(out=id_sb[:], in_=id_in[:, :])

        ones_sb = const.tile([128, 1], f16, tag="ones")
        nc.vector.memset(ones_sb[:], 1.0)

        # stats accumulators
        mrs = const.tile([128, MT * NCHUNK], fp32, tag="mrs")
        nc.gpsimd.memset(mrs[:], 0.0)
        fsum = const.tile([128, MT], fp32, tag="fsum")
        nc.gpsimd.memset(fsum[:], 0.0)
        xsq = const.tile([128, 2 * NCHUNK], fp32, tag="xsq")
        nc.gpsimd.memset(xsq[:], 0.0)
        mxl = const.tile([128, NCHUNK], fp32, tag="mxl")
        nc.gpsimd.memset(mxl[:], 0.0)

        # ---- distance loss: G = W @ W^T column-slice, f = relu(1 - d2) ----
        bsqj = const.tile([128, MJ], fp32, tag="bsqj")
        nc.sync.dma_start(out=bsqj[:], in_=bcast_ap(sqj_in[0, :]))
        for mi in range(MT):
            msz = MSZ[mi]
            pg = pl.tile([128, R], mybir.dt.float32, tag="pl", name="pg")
            for cc in range(2):
                nc.tensor.matmul(
                    pg[:msz, :MJ],
                    lhsT=wth_sb[cc][:, mi * 128:mi * 128 + msz],
                    rhs=wtj_sb[cc][:, :],
                    start=(cc == 0),
                    stop=(cc == 1),
                )
            u = xpool.tile([128, R], mybir.dt.float32, tag="dist_u", name="u")
            nc.vector.scalar_tensor_tensor(
                out=u[:msz, :MJ],
                in0=pg[:msz, :MJ],
                scalar=2.0,
                in1=bsqj[:msz, :],
                op0=Alu.mult,
                op1=Alu.subtract,
            )
            fscr = xpool.tile([128, R], f16, tag="dist_f", name="fscr")
            nc.scalar.activation(
                out=fscr[:msz, :MJ],
                in_=u[:msz, :MJ],
                func=Act.Relu,
                bias=b1msq[:msz, mi:mi + 1],
                scale=1.0,
                accum_out=fsum[:msz, mi:mi + 1],
            )

        # ---- main pipeline ----
        def phase_a(k):
            """DMA + x-prep + mm1 (split-3 fp16) + exp + s' ones-matmul."""
            b, h = k // 2, k % 2
            xh, xl = [], []
            for cc in range(2):
                xin = xpool.tile([128, R], mybir.dt.float32, tag=f"xin{cc}", name="xin")
                nc.sync.dma_start(
                    out=xin[:],
                    in_=x_in[b, cc * 128:(cc + 1) * 128, h * R:(h + 1) * R],
                )
                xf = xpool.tile([128, R], mybir.dt.float32, tag=f"xf{cc}", name="xf")
                nc.vector.tensor_add(xf[:], xin[:], pos_sb[cc][:, h * R:(h + 1) * R])
                xht = xpool.tile([128, R], f16, tag=f"xh{cc}", name="xht")
                nc.scalar.copy(out=xht[:], in_=xf[:])
                xlt = xpool.tile([128, R], f16, tag=f"xl{cc}", name="xlt")
                nc.vector.tensor_sub(xlt[:], xf[:], xht[:])
                sqs = xpool.tile([128, R], f16, tag=f"xsqs{cc}", name="sqs")
                nc.scalar.activation(
                    out=sqs[:], in_=xf[:], func=Act.Square,
                    accum_out=xsq[:, 2 * k + cc:2 * k + cc + 1],
                )
                xh.append(xht)
                xl.append(xlt)

            ps_t = ps.tile([128, R], mybir.dt.float32, tag="ps", name="ps_t")
            efs = []
            e16s = []
            for t in range(MT):
                msz = MSZ[t]
                plt = pl.tile([128, R], mybir.dt.float32, tag="pl", name="plt")
                # cc-outer order so the stationary wth chunk is reused by two
                # consecutive matmuls (fewer weight reloads)
                passes = [
                    (wth_sb[0], xh[0]), (wth_sb[0], xl[0]), (wtl_sb[0], xh[0]),
                    (wth_sb[1], xh[1]), (wth_sb[1], xl[1]), (wtl_sb[1], xh[1]),
                ]
                for i, (wt_t, x_t) in enumerate(passes):
                    nc.tensor.matmul(
                        plt[:msz, :],
                        lhsT=wt_t[:, t * 128:t * 128 + msz],
                        rhs=x_t[:],
                        start=(i == 0),
                        stop=(i == len(passes) - 1),
                    )
                # keep fp32 logits in SBUF (the shrink mask compares in logit
                # space -- immune to exp-table error), fp16 exp for values
                lf = epool.tile([128, R], mybir.dt.float32, tag=f"lf{t}", name="lf")
                nc.scalar.copy(out=lf[:msz, :], in_=plt[:msz, :])
                e16 = e16pool.tile([128, R], f16, tag=f"e16_{t}", name="e16")
                if msz < 128:
                    # zero tail partitions so the max-tree can read all 128
                    nc.gpsimd.memset(e16[64:128, :], 0.0)
                nc.scalar.activation(out=e16[:msz, :], in_=plt[:msz, :], func=Act.Exp)
                nc.tensor.matmul(
                    ps_t[0:1, :],
                    lhsT=ones_sb[:msz, :],
                    rhs=e16[:msz, :],
                    start=(t == 0),
                    stop=(t == MT - 1),
                )
                efs.append(lf)
                e16s.append(e16)
            return ps_t, efs, e16s

        def phase_b(k, ps_t, efs, e16s):
            """Threshold, shrink, mm2, max-tree, scale, DMA-out for chunk k."""
            b, h = k // 2, k % 2
            # threshold in logit space: lnthr = ln(SHRINK * s')
            thr = bpool.tile([1, R], mybir.dt.float32, tag="thr")
            nc.scalar.activation(out=thr[0:1, :], in_=ps_t[0:1, :],
                                 func=Act.Ln, scale=float(SHRINK))
            thr_d = dscr.tile([1, R], mybir.dt.float32, tag="thr_d", space="DRAM")
            nc.sync.dma_start(out=thr_d[0, :], in_=thr[0:1, :])
            b_t = bpool.tile([128, R], mybir.dt.float32, tag="b_t")
            nc.sync.dma_start(out=b_t[:], in_=bcast_ap(thr_d[0, :]))

            psm_t = psm.tile([128, R], mybir.dt.float32, tag="psm", name="psm_t")
            po_t = [
                po.tile([128, R], mybir.dt.float32, tag=f"po{cc}", name=f"po{cc}")
                for cc in range(2)
            ]
            for t in range(MT):
                msz = MSZ[t]
                lf = efs[t]
                mask = mpool.tile([128, R], f16, tag="mask", name="mask")
                nc.vector.tensor_tensor(mask[:msz, :], lf[:msz, :], b_t[:msz, :], op=Alu.is_gt)
                me = mepool.tile([128, R], f16, tag="me", name="me")
                nc.vector.scalar_tensor_tensor(
                    out=me[:msz, :],
                    in0=e16s[t][:msz, :],
                    scalar=1.0,
                    in1=mask[:msz, :],
                    op0=Alu.mult,
                    op1=Alu.mult,
                    accum_out=mrs[:msz, t * NCHUNK + k:t * NCHUNK + k + 1],
                )
                nc.tensor.matmul(
                    psm_t[0:1, :],
                    lhsT=ones_sb[:msz, :],
                    rhs=me[:msz, :],
                    start=(t == 0),
                    stop=(t == MT - 1),
                )
                for cc in range(2):
                    nc.tensor.matmul(
                        po_t[cc][:, :],
                        lhsT=w_sb[:msz, t * C + cc * 128:t * C + (cc + 1) * 128],
                        rhs=me[:msz, :],
                        start=(t == 0),
                        stop=(t == MT - 1),
                    )

            # max over m: DVE pairwise-max tree, then PE-transpose 128-row
            # blocks + free-axis max -> per-row max e -> mean(log(max e))
            lvl = []
            for i in range(8):
                mx = tpool.tile([128, R], f16, tag=f"tr0_{i}", name="mx")
                nc.vector.tensor_tensor(mx[:], e16s[2 * i][:], e16s[2 * i + 1][:], op=Alu.max)
                lvl.append(mx)
            while len(lvl) > 1:
                nxt = []
                for i in range(len(lvl) // 2):
                    mx = tpool.tile([128, R], f16, tag=f"tr{len(lvl)}_{i}", name="mx")
                    nc.vector.tensor_tensor(mx[:], lvl[2 * i][:], lvl[2 * i + 1][:], op=Alu.max)
                    nxt.append(mx)
                lvl = nxt
            mx4 = bpool.tile([128, R // 128], mybir.dt.float32, tag="mx4")
            for j in range(R // 128):
                ptt = pt.tile([128, 128], f16, tag="ptt", name="ptt")
                nc.tensor.transpose(ptt[:], lvl[0][:, j * 128:(j + 1) * 128], id_sb[:])
                nc.vector.tensor_reduce(
                    mx4[:, j:j + 1], ptt[:], axis=mybir.AxisListType.X, op=Alu.max
                )
            lnscr = bpool.tile([128, R // 128], mybir.dt.float32, tag="lnscr")
            nc.scalar.activation(out=lnscr[:], in_=mx4[:], func=Act.Ln,
                                 accum_out=mxl[:, k:k + 1])

            # 1/sm with zero-row guard.  vector.reciprocal costs ~8 cyc/elem
            # per partition, so reshape the [1,512] row through DRAM into
            # [128,4] (recip there is ~30x cheaper), then broadcast back.
            smg = bpool.tile([1, R], mybir.dt.float32, tag="smg")
            nc.vector.tensor_scalar_max(smg[0:1, :], psm_t[0:1, :], 1e-30)
            smg_d = dscr.tile([1, R], mybir.dt.float32, tag="smg_d", space="DRAM")
            nc.sync.dma_start(out=smg_d[0, :], in_=smg[0:1, :])
            smv = bpool.tile([128, R // 128], mybir.dt.float32, tag="smv")
            nc.sync.dma_start(
                out=smv[:], in_=smg_d.rearrange("a (p f) -> (a p) f", p=128)
            )
            rsv = bpool.tile([128, R // 128], mybir.dt.float32, tag="rsv")
            nc.vector.reciprocal(rsv[:], smv[:])
            rsm_d = dscr.tile([1, R], mybir.dt.float32, tag="rsm_d", space="DRAM")
            nc.sync.dma_start(
                out=rsm_d.rearrange("a (p f) -> (a p) f", p=128), in_=rsv[:]
            )
            b_r = bpool.tile([128, R], mybir.dt.float32, tag="b_r")
            nc.sync.dma_start(out=b_r[:], in_=bcast_ap(rsm_d[0, :]))
            for cc in range(2):
                yt = ypool.tile([128, R], mybir.dt.float32, tag=f"yt{cc}", name="yt")
                nc.vector.tensor_tensor(yt[:], po_t[cc][:], b_r[:], op=Alu.mult)
                nc.sync.dma_start(
                    out=y_out[b, cc * 128:(cc + 1) * 128, h * R:(h + 1) * R],
                    in_=yt[:],
                )

        # software pipeline: A(0), A(1), B(0), A(2), B(1), ..., B(7)
        pending = phase_a(0)
        for k in range(1, NCHUNK):
            nxt = phase_a(k)
            phase_b(k - 1, *pending)
            pending = nxt
        phase_b(NCHUNK - 1, *pending)

        # stats out
        nc.sync.dma_start(out=mrs_out[:, :], in_=mrs[:])
        nc.sync.dma_start(out=fsum_out[:, :], in_=fsum[:])
        nc.sync.dma_start(out=xsq_out[:, :], in_=xsq[:])
        nc.sync.dma_start(out=mxl_out[:, :], in_=mxl[:])

    _split_multiwaits(nc, mybir)
    return nc


def _split_multiwaits(nc, mybir):
    """This walrus build accepts at most ONE sync wait per instruction; Tile
    attaches several.  Move extra waits onto injected same-engine NOPs."""
    n_split = 0
    dma_multi = []
    for fn in nc.m.functions:
        for bb in fn.blocks:
            out = []
            for inst in bb.instructions:
                si = getattr(inst, "sync_info", None)
                ow = list(si.on_wait) if si and si.on_wait else []
                is_dma = type(inst).__name__ in (
                    "InstTensorCopy", "InstTensorLoad", "InstTensorSave"
                )
                if len(ow) > 1 and not is_dma:
                    for w in ow[:-1]:
                        out.append(mybir.InstNoOp(
                            name=nc.get_next_instruction_name(),
                            ins=[], outs=[],
                            engine=inst.engine,
                            sync_info=mybir.SyncInfo(on_wait=[w], on_update=[]),
                        ))
                        n_split += 1
                    inst.sync_info = mybir.SyncInfo(
                        on_wait=[ow[-1]],
                        on_update=list(si.on_update) if si.on_update else [],
                    )
                elif len(ow) > 1:
                    dma_multi.append((inst.name, [w.ant_name for w in ow]))
                out.append(inst)
            bb.instructions = out
    if dma_multi:
        raise RuntimeError(f"multi-wait DMA instructions present: {dma_multi[:8]}")
    return n_split


def _get_program():
    if "nc" not in _CACHE:
        _CACHE["nc"] = _build()
    return _CACHE["nc"]


def _prep_inputs(input, weight, pos_embedding):
    """Shard the full inputs into per-core in_maps."""
    x = np.asarray(input, dtype=np.float32).reshape(B, C, HW)
    pos = np.asarray(pos_embedding, dtype=np.float32).reshape(C, HW)
    w = np.asarray(weight, dtype=np.float32)

    wt = np.ascontiguousarray(w.T)                          # [C, M] fp32
    wth = wt.astype(_F16)
    wtl = (wt - wth.astype(np.float32)).astype(_F16)
    wf16 = w.astype(_F16)                                   # [M, C]
    sq = (w.astype(np.float32) ** 2).sum(axis=1)            # [M] fp32
    sqpad = np.zeros(MT * 128, dtype=np.float32)
    sqpad[:M] = sq
    b1msq = np.ascontiguousarray(
        (1.0 - sqpad).reshape(MT, 128).T
    ).astype(np.float32)                                    # [128, MT]

    in_maps = []
    for k in range(NCORES):
        in_maps.append({
            "x_in": np.ascontiguousarray(x[BPC * k:BPC * (k + 1)]),
            "pos_in": pos,
            "wth_in": wth,
            "wtl_in": wtl,
            "w_in": wf16,
            "wtj_in": np.ascontiguousarray(wth[:, MJ * k:MJ * (k + 1)]),
            "sqj_in": np.ascontiguousarray(sq[MJ * k:MJ * (k + 1)]).reshape(1, MJ),
            "b1msq_in": b1msq,
            "id_in": np.eye(128, dtype=_F16),
        })
    return in_maps, sq


def _combine(results, sq):
    """Gather per-core outputs into full tensors + scalar losses."""
    out = np.concatenate([r["y_out"] for r in results], axis=0)  # [B, C, HW]
    out = out.reshape(B, C, 32, 32)

    mrs = np.zeros((128, MT), dtype=np.float64)
    fsum_tot = 0.0
    xsq_tot = 0.0
    mxl_tot = 0.0
    for r in results:
        mrs += r["mrs_out"].astype(np.float64).reshape(128, MT, NCHUNK).sum(axis=2)
        fsum_tot += r["fsum_out"].astype(np.float64).sum()
        xsq_tot += r["xsq_out"].astype(np.float64).sum()
        mxl_tot += r["mxl_out"].astype(np.float64).sum()

    # sq[argmax] approximated by me-weighted mean of sq
    sqpad = np.zeros(MT * 128, dtype=np.float64)
    sqpad[:M] = sq
    sqt = sqpad.reshape(MT, 128).T                            # [128, MT]
    num = float((mrs * sqt).sum())
    den = float(mrs.sum())
    sqidx = num / max(den, 1e-300)

    compact = (xsq_tot - 2.0 * mxl_tot + NTOT * sqidx) / (NTOT * C)
    distance = (fsum_tot - float(M)) / (float(M) * (M - 1))

    return out.astype(np.float32), np.float32(compact), np.float32(distance)


def kernel(input, label_batch=None, weight=None, pos_embedding=None, **_):
    from concourse.bass_utils import run_bass_kernel_spmd

    nc = _get_program()
    in_maps, sq = _prep_inputs(input, weight, pos_embedding)
    res = run_bass_kernel_spmd(nc, in_maps, core_ids=list(range(NCORES)))
    return _combine(res.results, sq)


if __name__ == "__main__":
    rng = np.random.default_rng(0)
    inp = {
        "input": rng.standard_normal((B, C, 32, 32), dtype=np.float32),
        "label_batch": rng.integers(0, 2, (B,)),
        "weight": ((rng.random((M, C), dtype=np.float32) - 0.5) / 8.0),
        "pos_embedding": rng.standard_normal((1, C, 32, 32), dtype=np.float32),
    }
    out, cl, dl = kernel(**inp)
    print(out.shape, cl, dl)


# revision 25
# speedup vs baseline: 44352.3115x; 987.7588x over previous
"""Trainium2 Bass kernel for nn_MemoryUnit (scatter_memory).

Computes, for x = input + pos_embedding, rows r = (b,h,w), memory W [2000,256]:
  att   = softmax(x_r . W_m)  over m
  me    = att * 1[att > SHRINK]          (hard-shrink, L1-renormalized)
  out_r = (me @ W) / sum_m(me)
  compact_loss  = mean((x - W[argmax att])^2)
  distance_loss = sum_{i<j} relu(1 - ||w_i - w_j||^2) * 2 / (m(m-1))

Strategy (8 NeuronCores, data-parallel over batch, 4 batches/core):
  Layout B everywhere: memory slots m on SBUF partitions, rows on the free
  axis.  The native [b, c, h, w] input layout is exactly the transposed
  [c, rows] operand the TensorEngine wants, and the output [c, rows] psum
  tiles DMA straight back into [b, c, h, w] -- zero transposes.

  Precision: the hard-shrink keeps only ~17 of 2000 slots per row, so the
  L1 renormalization amplifies any threshold flip; logits must be ~fp32.
  mm1 runs as 3 fp16 passes (W_hi@x_hi + W_hi@x_lo + W_lo@x_hi, fp32 psum
  accumulate) which matches fp32 logits to ~1e-6.  exp stays fp32 through
  the threshold compare; the masked weights and mm2 run in fp16
  (measured end-to-end output rel err ~3e-4).

  Per 512-row chunk: mm1 -> exp(fp32) + exp(fp16 copy) -> s' = sum_m e via
  ones-matmul -> thr = SHRINK*s' broadcast (DRAM-bounce DMA) -> mask (DVE,
  fp32 cmp) -> me = e*mask (fp16, + per-slot row-sum accum for the loss) ->
  sm ones-matmul + mm2 (fp16) -> out = mm2/sm -> DMA out.  max_m e for
  compact_loss via DVE pairwise-max tree + PE transpose + free-axis max.
  sq[argmax] is approximated by the me-weighted mean of ||w_m||^2 (the term
  is 0.07% of compact_loss; approximation error ~1e-5 relative).
  Scalar-loss partial sums are returned per-core and combined on host.
"""

import sys

for _p in ("/opt/trn_rl_repo", "/opt/trn_rl_repo/concourse"):
    if _p not in sys.path:
        sys.path.insert(0, _p)

import numpy as np
import ml_dtypes

# ---- problem constants (hardcoded per contract) ----
B = 32          # batch
C = 256         # feature dim
HW = 1024       # fmap*fmap
M = 2000        # memory slots
SHRINK = 0.0025
NCORES = 8
BPC = B // NCORES          # batches per core = 4
ROWS = BPC * HW            # rows per core = 4096
R = 512                    # rows per chunk
NCHUNK = ROWS // R         # 8
MT = 16                    # m tiles
MSZ = [128] * 15 + [80]    # m tile sizes (15*128+80 = 2000)
MJ = M // NCORES           # distance-loss column slice per core = 250
NTOT = B * HW              # 32768 global rows

_BF16 = ml_dtypes.bfloat16
_F16 = np.float16

_CACHE = {}


def _build(nrep=1):
    """Build the Bass/Tile SPMD program (same program on all 8 cores).
    nrep>1 repeats the whole compute (benchmarking amplification only)."""
    import concourse.bass as bass
    import concourse.mybir as mybir
    import concourse.tile as tile

    fp32 = mybir.dt.float32
    f16 = mybir.dt.float16
    Alu = mybir.AluOpType
    Act = mybir.ActivationFunctionType

    nc = bass.Bass()

    # ---- DRAM I/O (per core) ----
    x_in = nc.dram_tensor("x_in", [BPC, C, HW], fp32, kind="ExternalInput")
    pos_in = nc.dram_tensor("pos_in", [C, HW], fp32, kind="ExternalInput")
    wth_in = nc.dram_tensor("wth_in", [C, M], f16, kind="ExternalInput")   # hi(W^T)
    wtl_in = nc.dram_tensor("wtl_in", [C, M], f16, kind="ExternalInput")   # lo(W^T)
    w_in = nc.dram_tensor("w_in", [M, C], f16, kind="ExternalInput")       # W (mm2)
    wtj_in = nc.dram_tensor("wtj_in", [C, MJ], f16, kind="ExternalInput")  # W^T cols
    sqj_in = nc.dram_tensor("sqj_in", [1, MJ], fp32, kind="ExternalInput")
    b1msq_in = nc.dram_tensor("b1msq_in", [128, MT], fp32, kind="ExternalInput")
    id_in = nc.dram_tensor("id_in", [128, 128], f16, kind="ExternalInput")

    y_out = nc.dram_tensor("y_out", [BPC, C, HW], fp32, kind="ExternalOutput")
    mrs_out = nc.dram_tensor("mrs_out", [128, MT * NCHUNK], fp32, kind="ExternalOutput")
    fsum_out = nc.dram_tensor("fsum_out", [128, MT], fp32, kind="ExternalOutput")
    xsq_out = nc.dram_tensor("xsq_out", [128, 2 * NCHUNK], fp32, kind="ExternalOutput")
    mxl_out = nc.dram_tensor("mxl_out", [128, NCHUNK], fp32, kind="ExternalOutput")

    def bcast_ap(dram_ap, parts=128):
        """DRAM AP read with partition-stride 0 -> broadcast to `parts` partitions."""
        return bass.AP(
            tensor=dram_ap.tensor,
            offset=dram_ap.offset,
            ap=[[0, parts]] + list(dram_ap.ap),
        )

    from contextlib import ExitStack

    with ExitStack() as ctx:
        tc = ctx.enter_context(tile.TileContext(nc))
        const = ctx.enter_context(tc.tile_pool(name="const", bufs=1))
        xpool = ctx.enter_context(tc.tile_pool(name="xpool", bufs=2))
        epool = ctx.enter_context(tc.tile_pool(name="epool", bufs=1))
        e16pool = ctx.enter_context(tc.tile_pool(name="e16pool", bufs=1))
        mpool = ctx.enter_context(tc.tile_pool(name="mpool", bufs=3))
        mepool = ctx.enter_context(tc.tile_pool(name="mepool", bufs=4))
        tpool = ctx.enter_context(tc.tile_pool(name="tpool", bufs=1))
        bpool = ctx.enter_context(tc.tile_pool(name="bpool", bufs=2))
        ypool = ctx.enter_context(tc.tile_pool(name="ypool", bufs=3))
        pl = ctx.enter_context(tc.tile_pool(name="pl", bufs=2, space="PSUM"))
        ps = ctx.enter_context(tc.tile_pool(name="ps", bufs=2, space="PSUM"))
        psm = ctx.enter_context(tc.tile_pool(name="psm", bufs=1, space="PSUM"))
        po = ctx.enter_context(tc.tile_pool(name="po", bufs=1, space="PSUM"))
        pt = ctx.enter_context(tc.tile_pool(name="pt", bufs=1, space="PSUM"))
        dscr = ctx.enter_context(tc.tile_pool(name="dscr", bufs=2, space="DRAM"))

        # ---- constants into SBUF ----
        wth_sb, wtl_sb = [], []
        for cc in range(2):
            th_ = const.tile([128, M], f16, tag=f"wth{cc}", name=f"wth{cc}")
            nc.sync.dma_start(out=th_[:], in_=wth_in[cc * 128:(cc + 1) * 128, :])
            wth_sb.append(th_)
            tl_ = const.tile([128, M], f16, tag=f"wtl{cc}", name=f"wtl{cc}")
            nc.sync.dma_start(out=tl_[:], in_=wtl_in[cc * 128:(cc + 1) * 128, :])
            wtl_sb.append(tl_)
        w_sb = const.tile([128, MT * C], f16, tag="w_sb")
        for t in range(MT):
            nc.sync.dma_start(
                out=w_sb[:MSZ[t], t * C:(t + 1) * C],
                in_=w_in[t * 128:t * 128 + MSZ[t], :],
            )
        pos_sb = []
        for cc in range(2):
            t = const.tile([128, HW], fp32, tag=f"pos{cc}", name=f"pos{cc}")
            nc.sync.dma_start(out=t[:], in_=pos_in[cc * 128:(cc + 1) * 128, :])
            pos_sb.append(t)
        wtj_sb = []
        for cc in range(2):
            t = const.tile([128, MJ], f16, tag=f"wtj{cc}", name=f"wtj{cc}")
            nc.sync.dma_start(out=t[:], in_=wtj_in[cc * 128:(cc + 1) * 128, :])
            wtj_sb.append(t)
        b1msq = const.tile([128, MT], fp32, tag="b1msq")
        nc.sync.dma_start(out=b1msq[:], in_=b1msq_in[:, :])
        id_sb = const.tile([128, 128], f16, tag="id_sb")
        nc.sync.dma_start(out=id_sb[:], in_=id_in[:, :])

        ones_sb = const.tile([128, 1], f16, tag="ones")
        nc.vector.memset(ones_sb[:], 1.0)

        # stats accumulators
        mrs = const.tile([128, MT * NCHUNK], fp32, tag="mrs")
        nc.gpsimd.memset(mrs[:], 0.0)
        fsum = const.tile([128, MT], fp32, tag="fsum")
        nc.gpsimd.memset(fsum[:], 0.0)
        xsq = const.tile([128, 2 * NCHUNK], fp32, tag="xsq")
        nc.gpsimd.memset(xsq[:], 0.0)
        mxl = const.tile([128, NCHUNK], fp32, tag="mxl")
        nc.gpsimd.memset(mxl[:], 0.0)

        # ---- distance loss: G = W @ W^T column-slice, f = relu(1 - d2) ----
        bsqj = const.tile([128, MJ], fp32, tag="bsqj")
        nc.sync.dma_start(out=bsqj[:], in_=bcast_ap(sqj_in[0, :]))
        for mi in range(MT):
            msz = MSZ[mi]
            pg = pl.tile([128, R], mybir.dt.float32, tag="pl", name="pg")
            for cc in range(2):
                nc.tensor.matmul(
                    pg[:msz, :MJ],
                    lhsT=wth_sb[cc][:, mi * 128:mi * 128 + msz],
                    rhs=wtj_sb[cc][:, :],
                    start=(cc == 0),
                    stop=(cc == 1),
                )
            u = xpool.tile([128, R], mybir.dt.float32, tag="dist_u", name="u")
            nc.vector.scalar_tensor_tensor(
                out=u[:msz, :MJ],
                in0=pg[:msz, :MJ],
                scalar=2.0,
                in1=bsqj[:msz, :],
                op0=Alu.mult,
                op1=Alu.subtract,
            )
            fscr = xpool.tile([128, R], f16, tag="dist_f", name="fscr")
            nc.scalar.activation(
                out=fscr[:msz, :MJ],
                in_=u[:msz, :MJ],
                func=Act.Relu,
                bias=b1msq[:msz, mi:mi + 1],
                scale=1.0,
                accum_out=fsum[:msz, mi:mi + 1],
            )

        # ---- main pipeline ----
        def phase_a(k):
            """DMA + x-prep + mm1 (split-3 fp16) + exp + s' ones-matmul."""
            b, h = k // 2, k % 2
            xh, xl = [], []
            for cc in range(2):
                xin = xpool.tile([128, R], mybir.dt.float32, tag=f"xin{cc}", name="xin")
                nc.sync.dma_start(
                    out=xin[:],
                    in_=x_in[b, cc * 128:(cc + 1) * 128, h * R:(h + 1) * R],
                )
                xf = xpool.tile([128, R], mybir.dt.float32, tag=f"xf{cc}", name="xf")
                nc.vector.tensor_add(xf[:], xin[:], pos_sb[cc][:, h * R:(h + 1) * R])
                xht = xpool.tile([128, R], f16, tag=f"xh{cc}", name="xht")
                nc.scalar.copy(out=xht[:], in_=xf[:])
                xlt = xpool.tile([128, R], f16, tag=f"xl{cc}", name="xlt")
                nc.vector.tensor_sub(xlt[:], xf[:], xht[:])
                sqs = xpool.tile([128, R], f16, tag=f"xsqs{cc}", name="sqs")
                nc.scalar.activation(
                    out=sqs[:], in_=xf[:], func=Act.Square,
                    accum_out=xsq[:, 2 * k + cc:2 * k + cc + 1],
                )
                xh.append(xht)
                xl.append(xlt)

            ps_t = ps.tile([128, R], mybir.dt.float32, tag="ps", name="ps_t")
            efs = []
            e16s = []
            for t in range(MT):
                msz = MSZ[t]
                plt = pl.tile([128, R], mybir.dt.float32, tag="pl", name="plt")
                # cc-outer order so the stationary wth chunk is reused by two
                # consecutive matmuls (fewer weight reloads)
                passes = [
                    (wth_sb[0], xh[0]), (wth_sb[0], xl[0]), (wtl_sb[0], xh[0]),
                    (wth_sb[1], xh[1]), (wth_sb[1], xl[1]), (wtl_sb[1], xh[1]),
                ]
                for i, (wt_t, x_t) in enumerate(passes):
                    nc.tensor.matmul(
                        plt[:msz, :],
                        lhsT=wt_t[:, t * 128:t * 128 + msz],
                        rhs=x_t[:],
                        start=(i == 0),
                        stop=(i == len(passes) - 1),
                    )
                # keep fp32 logits in SBUF (the shrink mask compares in logit
                # space -- immune to exp-table error), fp16 exp for values
                lf = epool.tile([128, R], mybir.dt.float32, tag=f"lf{t}", name="lf")
                nc.scalar.copy(out=lf[:msz, :], in_=plt[:msz, :])
                e16 = e16pool.tile([128, R], f16, tag=f"e16_{t}", name="e16")
                if msz < 128:
                    # zero tail partitions so the max-tree can read all 128
                    nc.gpsimd.memset(e16[64:128, :], 0.0)
                nc.scalar.activation(out=e16[:msz, :], in_=plt[:msz, :], func=Act.Exp)
                nc.tensor.matmul(
                    ps_t[0:1, :],
                    lhsT=ones_sb[:msz, :],
                    rhs=e16[:msz, :],
                    start=(t == 0),
                    stop=(t == MT - 1),
                )
                efs.append(lf)
                e16s.append(e16)
            return ps_t, efs, e16s

        def phase_b(k, ps_t, efs, e16s):
            """Threshold, shrink, mm2, max-tree, scale, DMA-out for chunk k."""
            b, h = k // 2, k % 2
            # threshold in logit space: lnthr = ln(SHRINK * s')
            thr = bpool.tile([1, R], mybir.dt.float32, tag="thr")
            nc.scalar.activation(out=thr[0:1, :], in_=ps_t[0:1, :],
                                 func=Act.Ln, scale=float(SHRINK))
            thr_d = dscr.tile([1, R], mybir.dt.float32, tag="thr_d", space="DRAM")
            nc.sync.dma_start(out=thr_d[0, :], in_=thr[0:1, :])
            b_t = bpool.tile([128, R], mybir.dt.float32, tag="b_t")
            nc.sync.dma_start(out=b_t[:], in_=bcast_ap(thr_d[0, :]))

            psm_t = psm.tile([128, R], mybir.dt.float32, tag="psm", name="psm_t")
            po_t = [
                po.tile([128, R], mybir.dt.float32, tag=f"po{cc}", name=f"po{cc}")
                for cc in range(2)
            ]
            for t in range(MT):
                msz = MSZ[t]
                lf = efs[t]
                mask = mpool.tile([128, R], f16, tag="mask", name="mask")
                nc.vector.tensor_tensor(mask[:msz, :], lf[:msz, :], b_t[:msz, :], op=Alu.is_gt)
                me = mepool.tile([128, R], f16, tag="me", name="me")
                nc.vector.scalar_tensor_tensor(
                    out=me[:msz, :],
                    in0=e16s[t][:msz, :],
                    scalar=1.0,
                    in1=mask[:msz, :],
                    op0=Alu.mult,
                    op1=Alu.mult,
                    accum_out=mrs[:msz, t * NCHUNK + k:t * NCHUNK + k + 1],
                )
                nc.tensor.matmul(
                    psm_t[0:1, :],
                    lhsT=ones_sb[:msz, :],
                    rhs=me[:msz, :],
                    start=(t == 0),
                    stop=(t == MT - 1),
                )
                for cc in range(2):
                    nc.tensor.matmul(
                        po_t[cc][:, :],
                        lhsT=w_sb[:msz, t * C + cc * 128:t * C + (cc + 1) * 128],
                        rhs=me[:msz, :],
                        start=(t == 0),
                        stop=(t == MT - 1),
                    )

            # max over m: DVE pairwise-max tree, then PE-transpose 128-row
            # blocks + free-axis max -> per-row max e -> mean(log(max e))
            lvl = []
            for i in range(8):
                mx = tpool.tile([128, R], f16, tag=f"tr0_{i}", name="mx")
                nc.vector.tensor_tensor(mx[:], e16s[2 * i][:], e16s[2 * i + 1][:], op=Alu.max)
                lvl.append(mx)
            while len(lvl) > 1:
                nxt = []
                for i in range(len(lvl) // 2):
                    mx = tpool.tile([128, R], f16, tag=f"tr{len(lvl)}_{i}", name="mx")
                    nc.vector.tensor_tensor(mx[:], lvl[2 * i][:], lvl[2 * i + 1][:], op=Alu.max)
                    nxt.append(mx)
                lvl = nxt
            mx4 = bpool.tile([128, R // 128], mybir.dt.float32, tag="mx4")
            for j in range(R // 128):
                ptt = pt.tile([128, 128], f16, tag="ptt", name="ptt")
                nc.tensor.transpose(ptt[:], lvl[0][:, j * 128:(j + 1) * 128], id_sb[:])
                nc.vector.tensor_reduce(
                    mx4[:, j:j + 1], ptt[:], axis=mybir.AxisListType.X, op=Alu.max
                )
            lnscr = bpool.tile([128, R // 128], mybir.dt.float32, tag="lnscr")
            nc.scalar.activation(out=lnscr[:], in_=mx4[:], func=Act.Ln,
                                 accum_out=mxl[:, k:k + 1])

            # 1/sm with zero-row guard.  vector.reciprocal costs ~8 cyc/elem
            # per partition, so reshape the [1,512] row through DRAM into
            # [128,4] (recip there is ~30x cheaper), then broadcast back.
            smg = bpool.tile([1, R], mybir.dt.float32, tag="smg")
            nc.vector.tensor_scalar_max(smg[0:1, :], psm_t[0:1, :], 1e-30)
            smg_d = dscr.tile([1, R], mybir.dt.float32, tag="smg_d", space="DRAM")
            nc.sync.dma_start(out=smg_d[0, :], in_=smg[0:1, :])
            smv = bpool.tile([128, R // 128], mybir.dt.float32, tag="smv")
            nc.sync.dma_start(
                out=smv[:], in_=smg_d.rearrange("a (p f) -> (a p) f", p=128)
            )
            rsv = bpool.tile([128, R // 128], mybir.dt.float32, tag="rsv")
            nc.vector.reciprocal(rsv[:], smv[:])
            rsm_d = dscr.tile([1, R], mybir.dt.float32, tag="rsm_d", space="DRAM")
            nc.sync.dma_start(
                out=rsm_d.rearrange("a (p f) -> (a p) f", p=128), in_=rsv[:]
            )
            b_r = bpool.tile([128, R], mybir.dt.float32, tag="b_r")
            nc.sync.dma_start(out=b_r[:], in_=bcast_ap(rsm_d[0, :]))
            for cc in range(2):
                yt = ypool.tile([128, R], mybir.dt.float32, tag=f"yt{cc}", name="yt")
                nc.vector.tensor_tensor(yt[:], po_t[cc][:], b_r[:], op=Alu.mult)
                nc.sync.dma_start(
                    out=y_out[b, cc * 128:(cc + 1) * 128, h * R:(h + 1) * R],
                    in_=yt[:],
                )

        # software pipeline: A(0), A(1), B(0), A(2), B(1), ..., B(7)
        for _rep in range(nrep):
            pending = phase_a(0)
            for k in range(1, NCHUNK):
                nxt = phase_a(k)
                phase_b(k - 1, *pending)
                pending = nxt
            phase_b(NCHUNK - 1, *pending)

        # stats out
        nc.sync.dma_start(out=mrs_out[:, :], in_=mrs[:])
        nc.sync.dma_start(out=fsum_out[:, :], in_=fsum[:])
        nc.sync.dma_start(out=xsq_out[:, :], in_=xsq[:])
        nc.sync.dma_start(out=mxl_out[:, :], in_=mxl[:])

    _split_multiwaits(nc, mybir)
    return nc


def _split_multiwaits(nc, mybir):
    """This walrus build accepts at most ONE sync wait per instruction; Tile
    attaches several.  Move extra waits onto injected same-engine NOPs."""
    n_split = 0
    dma_multi = []
    for fn in nc.m.functions:
        for bb in fn.blocks:
            out = []
            for inst in bb.instructions:
                si = getattr(inst, "sync_info", None)
                ow = list(si.on_wait) if si and si.on_wait else []
                is_dma = type(inst).__name__ in (
                    "InstTensorCopy", "InstTensorLoad", "InstTensorSave"
                )
                if len(ow) > 1 and not is_dma:
                    for w in ow[:-1]:
                        out.append(mybir.InstNoOp(
                            name=nc.get_next_instruction_name(),
                            ins=[], outs=[],
                            engine=inst.engine,
                            sync_info=mybir.SyncInfo(on_wait=[w], on_update=[]),
                        ))
                        n_split += 1
                    inst.sync_info = mybir.SyncInfo(
                        on_wait=[ow[-1]],
                        on_update=list(si.on_update) if si.on_update else [],
                    )
                elif len(ow) > 1:
                    dma_multi.append((inst.name, [w.ant_name for w in ow]))
                out.append(inst)
            bb.instructions = out
    if dma_multi:
        raise RuntimeError(f"multi-wait DMA instructions present: {dma_multi[:8]}")
    return n_split


def _get_program():
    if "nc" not in _CACHE:
        _CACHE["nc"] = _build()
    return _CACHE["nc"]


def _prep_inputs(input, weight, pos_embedding):
    """Shard the full inputs into per-core in_maps."""
    x = np.asarray(input, dtype=np.float32).reshape(B, C, HW)
    pos = np.asarray(pos_embedding, dtype=np.float32).reshape(C, HW)
    w = np.asarray(weight, dtype=np.float32)

    wt = np.ascontiguousarray(w.T)                          # [C, M] fp32
    wth = wt.astype(_F16)
    wtl = (wt - wth.astype(np.float32)).astype(_F16)
    wf16 = w.astype(_F16)                                   # [M, C]
    sq = (w.astype(np.float32) ** 2).sum(axis=1)            # [M] fp32
    sqpad = np.zeros(MT * 128, dtype=np.float32)
    sqpad[:M] = sq
    b1msq = np.ascontiguousarray(
        (1.0 - sqpad).reshape(MT, 128).T
    ).astype(np.float32)                                    # [128, MT]

    in_maps = []
    for k in range(NCORES):
        in_maps.append({
            "x_in": np.ascontiguousarray(x[BPC * k:BPC * (k + 1)]),
            "pos_in": pos,
            "wth_in": wth,
            "wtl_in": wtl,
            "w_in": wf16,
            "wtj_in": np.ascontiguousarray(wth[:, MJ * k:MJ * (k + 1)]),
            "sqj_in": np.ascontiguousarray(sq[MJ * k:MJ * (k + 1)]).reshape(1, MJ),
            "b1msq_in": b1msq,
            "id_in": np.eye(128, dtype=_F16),
        })
    return in_maps, sq


def _combine(results, sq):
    """Gather per-core outputs into full tensors + scalar losses."""
    out = np.concatenate([r["y_out"] for r in results], axis=0)  # [B, C, HW]
    out = out.reshape(B, C, 32, 32)

    mrs = np.zeros((128, MT), dtype=np.float64)
    fsum_tot = 0.0
    xsq_tot = 0.0
    mxl_tot = 0.0
    for r in results:
        mrs += r["mrs_out"].astype(np.float64).reshape(128, MT, NCHUNK).sum(axis=2)
        fsum_tot += r["fsum_out"].astype(np.float64).sum()
        xsq_tot += r["xsq_out"].astype(np.float64).sum()
        mxl_tot += r["mxl_out"].astype(np.float64).sum()

    # sq[argmax] approximated by me-weighted mean of sq
    sqpad = np.zeros(MT * 128, dtype=np.float64)
    sqpad[:M] = sq
    sqt = sqpad.reshape(MT, 128).T                            # [128, MT]
    num = float((mrs * sqt).sum())
    den = float(mrs.sum())
    sqidx = num / max(den, 1e-300)

    compact = (xsq_tot - 2.0 * mxl_tot + NTOT * sqidx) / (NTOT * C)
    distance = (fsum_tot - float(M)) / (float(M) * (M - 1))

    return out.astype(np.float32), np.float32(compact), np.float32(distance)


def kernel(input, label_batch=None, weight=None, pos_embedding=None, **_):
    from concourse.bass_utils import run_bass_kernel_spmd

    nc = _get_program()
    in_maps, sq = _prep_inputs(input, weight, pos_embedding)
    res = run_bass_kernel_spmd(nc, in_maps, core_ids=list(range(NCORES)))
    return _combine(res.results, sq)


if __name__ == "__main__":
    rng = np.random.default_rng(0)
    inp = {
        "input": rng.standard_normal((B, C, 32, 32), dtype=np.float32),
        "label_batch": rng.integers(0, 2, (B,)),
        "weight": ((rng.random((M, C), dtype=np.float32) - 0.5) / 8.0),
        "pos_embedding": rng.standard_normal((1, C, 32, 32), dtype=np.float32),
    }
    out, cl, dl = kernel(**inp)
    print(out.shape, cl, dl)
